# revision 18
# baseline (speedup 1.0000x reference)
"""Multi-head attention (RoPE) Trainium2 kernel.

Problem: B=2, T=2048, D_MODEL=1024, 16 heads x d_k=64, fp32 in/out.

Sharding: tensor-parallel over heads. Core c owns heads 2c, 2c+1:
  - wq/wk/wv rows [128c, 128c+128)  (column-split of the projections)
  - wo columns [128c, 128c+128)     (row-split of the output projection)
Each core emits a NORMALIZED fp16 partial of the output projection for its
two heads; the host sums the 8 partials (the "all-reduce" of row-parallel wo).

On-chip dataflow per core (fp16 matmul operands, fp32 PSUM):
  xT [D=1024, tok=4096] (token-major b*2048+s) @ wT slices -> QT/KT/VT [128, 4096]
  RoPE on QT/KT in [d', tok] layout (tables precomputed host-side, partition
  swap via SBUF-SBUF DMA).
  V transposed per 128-token tile on the PE to [tok, 64]-per-head tiles with
  a ones column appended (the 65th stationary column makes the AV matmul
  accumulate the softmax denominator into PSUM row 64 for free).
  Attention runs in 512-query chunks. Per key tile kt, the two heads' score
  matmuls are row-tiled ((0,0)/(64,0)) and run CONCURRENTLY on the PE,
  writing the two bank-halves of one [128,1024] PSUM tile, so a single
  ACTIVATE (exp, scale=1/8 folded; no max-subtraction: scores ~ N(0,1))
  covers both heads at the full-rate (N+352)/1.2 ns cost.
  Normalization on-device: den rows are broadcast across the 64 head
  partitions with contraction-1 matmuls into one aux PSUM tile, inverted
  with the lane-parallel fast custom-DVE reciprocal (requires base partition
  0), and multiplied into the O^T eviction. The output projection merges
  both heads (contraction 128) into one fp16 partial summed across cores.

Scheduling: the scalar engine's exp stream (128 x ACTIVATE[128,1024] per
core ~ 147us) is the kernel floor, and the PE total (~145us) ties it, so
both engines must stay saturated. PSUM layout is the key: tag "sc"
2x[128,1024] (4 banks) is used ONLY by score tiles, so their double-buffer
rotation depends only on ACT; tag "o" 2x[65,512] (2 banks) holds the AV
accumulators; tag "aux" 1x[128,1024] (2 banks) serves ALL other PSUM users
(projections, V-transposes, den broadcast, output projection), which chain
among themselves without ever blocking a score tile. Projection / RoPE /
V-transpose / output-projection work is emitted as self-contained "filler"
pieces inside the attention kt loops (the Tile scheduler is dependency-
driven, so emission position only shapes the PSUM rotation and priorities).
Each chunk's normalization + output projection run as fillers of the NEXT
chunk. x slices are DMA'd lazily so RoPE-swap DMAs aren't queued behind
megabytes of input traffic.
"""

import sys

sys.path.insert(0, "/opt/trn_rl_repo")

import numpy as np

import concourse.bacc as bacc
import concourse.bass as bass
import concourse.tile as tile
from concourse import mybir
from concourse.masks import make_identity

F16 = mybir.dt.float16
F32 = mybir.dt.float32

B = 2
T = 2048
D = 1024
NTOK = B * T  # 4096
DK = 64
N_CORES = 8
QCH = 512  # query chunk
KT_N = T // 128  # 16 key tiles per batch


def _build_body(tc, xT, wqT, wkT, wvT, woT, ropeA, ropeB, outT):
    nc = tc.nc
    Exp = mybir.ActivationFunctionType.Exp

    const = tc.alloc_tile_pool(name="const", bufs=1)
    psum = tc.alloc_tile_pool(name="psum", bufs=1, space="PSUM")

    # ---------------- persistent tiles ----------------
    w_sb = {}
    wkt = const.tile([128, 8, 128], F16, name="wksb")
    nc.scalar.dma_start(out=wkt, in_=wkT.rearrange("(a p) m -> p a m", p=128))
    w_sb["wk"] = wkt
    wqt = const.tile([128, 8, 128], F16, name="wqsb")

    xs = [const.tile([128, 4096], F16, name=f"xs{k}") for k in range(8)]

    def load_xs(t4, half=None):
        cs = slice(t4 * 1024, (t4 + 1) * 1024)
        if half is not None:
            cs = slice(t4 * 1024 + half * 512, t4 * 1024 + half * 512 + 512)
        for k in range(8):
            nc.sync.dma_start(out=xs[k][:, cs], in_=xT[k * 128 : (k + 1) * 128, cs])

    load_xs(0, half=0)
    # remaining const loads (rope tables, wv, wo) are emitted further down,
    # interleaved with the prelude pieces that consume them: a consumer
    # waits for ALL earlier DMAs on its queue (watermark semantics), so
    # queue order must match consumption order.
    rA = const.tile([128, 2048], F16)
    rB = const.tile([128, 2048], F16)
    wv_t = const.tile([128, 8, 128], F16, name="wvsb")
    wo_sb = const.tile([128, 1024], F16)
    ident = const.tile([128, 128], F16)
    make_identity(nc, ident)
    ones_sb = const.tile([128, 64], F16)
    nc.vector.memset(ones_sb, 1.0)

    q_rot = const.tile([128, 4096], F16)
    k_rot = const.tile([128, 4096], F16)
    # per 128-token tile, per head: [V(0:64) | ones(64) | pad] fp16
    v_sb = [
        [const.tile([128, 72], F16, name=f"vsb{i}h{h}") for h in range(2)]
        for i in range(NTOK // 128)
    ]
    for vpair in v_sb:
        for vt in vpair:
            nc.vector.memset(vt, 1.0)

    at = tc.alloc_tile_pool(name="attn", bufs=1)
    pp = tc.alloc_tile_pool(name="phasep", bufs=1)

    vt_raw = pp.tile([128, 4096], F16)

    # ------------- phase P pieces (fillers; all PSUM via tag "aux") ---------
    def proj_chunk(nm, dst, toff, width=1024, tag="aux"):
        wt = w_sb[nm]
        ps = psum.tile([128, 1024], F32, tag=tag, bufs=1 if tag == "aux" else 2, name="ps_pr")
        nh = width // 512
        for k in range(8):
            for h2 in range(nh):
                nc.tensor.matmul(
                    ps[:, h2 * 512 : (h2 + 1) * 512],
                    lhsT=wt[:, k, :],
                    rhs=xs[k][:, toff + h2 * 512 : toff + (h2 + 1) * 512],
                    start=(k == 0),
                    stop=(k == 7),
                )
        nc.vector.tensor_copy(dst[:, toff : toff + width], ps[:, 0:width])

    def rope_chunk(raw, toff, width=1024):
        # out = raw*A + swap(raw)*B, swap = +-32 partitions within a head
        cs = slice(toff, toff + width)
        rs = slice(toff % 2048, toff % 2048 + width)
        sw = pp.tile([128, 1024], F16, tag="sw", bufs=2, name="ropesw")
        for dst_p, src_p in ((0, 32), (32, 0), (64, 96), (96, 64)):
            nc.sync.dma_start(
                out=sw[dst_p : dst_p + 32, 0:width], in_=raw[src_p : src_p + 32, cs]
            )
        t1 = pp.tile([128, 1024], F16, tag="t1", bufs=2, name="ropet1")
        nc.vector.tensor_mul(t1[:, 0:width], raw[:, cs], rA[:, rs])
        nc.vector.tensor_mul(sw[:, 0:width], sw[:, 0:width], rB[:, rs])
        nc.vector.tensor_add(raw[:, cs], t1[:, 0:width], sw[:, 0:width])

    def v_chunk_transpose(t4, tag="aux"):
        # V transpose on the PE: vt_raw [d', tok] -> v_sb [tok128, d64]
        # 8 transposes share ONE tile (f16 views of its 8 64-col strips)
        pst = psum.tile([128, 1024], F32, tag=tag, bufs=1 if tag == "aux" else 2, name="ps_tr")
        for j, i in enumerate(range(8 * t4, 8 * (t4 + 1))):
            ts = slice(i * 128, (i + 1) * 128)
            tr = pst[:, j * 64 : j * 64 + 64].bitcast(F16)  # [128, 128] f16 view
            nc.tensor.transpose(tr, vt_raw[:, ts], ident)
            nc.vector.tensor_copy(v_sb[i][0][:, 0:64], tr[:, 0:64])
            nc.vector.tensor_copy(v_sb[i][1][:, 0:64], tr[:, 64:128])

    # ---------------- attention ----------------
    # Flat global step loop: one step per (chunk, kt). The score/exp stream
    # is emitted with a 2-step lookahead that CROSSES chunk boundaries, so
    # the ACT queue never drains at a boundary; AV, normalization, output
    # projection and phase-P pieces trail behind as fillers.
    chunks = [(b, qh) for b in (0, 1) for qh in range(4)]
    NC_ = len(chunks)
    exp_tiles = {}
    ps_os = {}
    ocats = {}

    def qoff_of(ci):
        b, qh = chunks[ci]
        return b * T + qh * QCH

    def s_exp(ci, kt):
        # two heads' score MMs run concurrently (row-tiled), writing the two
        # bank-halves of one [128,1024] tile -> a single ACTIVATE
        b, qh = chunks[ci]
        qoff = qoff_of(ci)
        koff = b * T + kt * 128
        ps = psum.tile([128, 1024], F32, tag="sc", bufs=2, name="ps_s")
        for hi in range(2):
            hs = slice(64 * hi, 64 * hi + 64)
            nc.tensor.matmul(
                ps[:, hi * 512 : (hi + 1) * 512],
                lhsT=k_rot[hs, koff : koff + 128],
                rhs=q_rot[hs, qoff : qoff + QCH],
                start=True,
                stop=True,
            )
        e = at.tile([128, 1024], F16, tag="exp", bufs=16, name="exps")
        nc.scalar.activation(e, ps, Exp, scale=0.125)
        exp_tiles[(ci, kt)] = e

    def av(ci, kt):
        b, qh = chunks[ci]
        if kt == 0:
            ps_os[ci] = [
                psum.tile([65, 512], F32, tag="o", bufs=2, name=f"ps_o{hi}")
                for hi in range(2)
            ]
        ps_o = ps_os[ci]
        vt = v_sb[b * KT_N + kt]
        e = exp_tiles.pop((ci, kt))
        for hi in range(2):
            nc.tensor.matmul(
                ps_o[hi],
                lhsT=vt[hi][:, 0:65],
                rhs=e[:, hi * 512 : (hi + 1) * 512],
                start=(kt == 0),
                stop=(kt == KT_N - 1),
                skip_group_check=True,
            )

    def make_norm(ci):
        # part 1 (now): evict den rows (PSUM row 64 -> SBUF).
        ps_o = ps_os[ci]
        den_row = at.tile([128, 1024], F16, tag="den", bufs=2, name="den_row")
        for hi in range(2):
            nc.vector.tensor_copy(
                den_row[64:65, hi * 512 : (hi + 1) * 512], ps_o[hi][64:65, :]
            )
        ocat = at.tile([128, 512], F16, tag="ocat", bufs=2, name="ocat")
        oBt = at.tile([64, 512], F16, tag="oBt", bufs=2, name="oBt")
        ocats[ci] = ocat

        def norm():
            # part 2 (as a filler): broadcast den across the 64 head
            # partitions (contraction-1 MMs, operands on partition 64) into
            # one aux tile, lane-parallel fast reciprocal at base partition 0
            # (custom-DVE ops no-op at base > 0), then scale the eviction.
            rb_ps = psum.tile([128, 1024], F32, tag="aux", bufs=1, name="rb_ps")
            for hi in range(2):
                nc.tensor.matmul(
                    rb_ps[0:64, hi * 512 : (hi + 1) * 512],
                    lhsT=ones_sb[64:65, :],
                    rhs=den_row[64:65, hi * 512 : (hi + 1) * 512],
                    start=True,
                    stop=True,
                )
            rb = at.tile([64, 1024], F32, tag="rb", bufs=2, name="rb")
            nc.vector.reciprocal_approx_fast(rb, rb_ps[0:64, :])
            nc.vector.tensor_mul(ocat[0:64, :], ps_o[0][0:64, :], rb[:, 0:512])
            nc.vector.tensor_mul(oBt, ps_o[1][0:64, :], rb[:, 512:1024])
            nc.sync.dma_start(out=ocat[64:128, :], in_=oBt)

        return norm

    def oproj_piece(ci, j, tag="aux"):
        # merged output projection (contraction 128 over both heads)
        qoff = qoff_of(ci)

        def run():
            ps_u = psum.tile(
                [128, 1024], F32, tag=tag, bufs=1 if tag == "aux" else 2, name="ps_u"
            )
            ocat = ocats[ci]
            for i in range(2):
                nt = 2 * j + i
                nc.tensor.matmul(
                    ps_u[:, i * 512 : (i + 1) * 512],
                    lhsT=wo_sb[:, nt * 128 : (nt + 1) * 128],
                    rhs=ocat,
                    start=True,
                    stop=True,
                )
            ot = at.tile([128, 1024], F16, tag="ot", bufs=2, name="ot")
            nc.vector.tensor_copy(ot, ps_u)
            for i in range(2):
                nt = 2 * j + i
                nc.sync.dma_start(
                    out=outT[nt * 128 : (nt + 1) * 128, qoff : qoff + QCH],
                    in_=ot[:, i * 512 : (i + 1) * 512],
                )

        return run

    # prelude: k (both halves of batch 0), q for the first 512 queries, and
    # V tiles 0-7; everything else streams in as 512-wide filler pieces
    # (each ~1.75us of PE time, under the 2-tile ACT backlog).
    def pj(nm, dst, toff):
        return lambda: proj_chunk(nm, dst, toff, 512)

    def rp(raw, toff):
        return lambda: rope_chunk(raw, toff, 512)

    proj_chunk("wk", k_rot, 0, 512, tag="sc")
    nc.scalar.dma_start(out=wqt, in_=wqT.rearrange("(a p) m -> p a m", p=128))
    w_sb["wq"] = wqt
    load_xs(0, half=1)
    proj_chunk("wk", k_rot, 512, 512, tag="aux")
    proj_chunk("wq", q_rot, 0, width=512, tag="sc")
    nc.sync.dma_start(out=rA, in_=ropeA)
    nc.sync.dma_start(out=rB, in_=ropeB)
    for toff in (0, 512):
        rope_chunk(k_rot, toff, 512)
    rope_chunk(q_rot, 0, width=512)
    nc.sync.dma_start(out=wv_t, in_=wvT.rearrange("(a p) m -> p a m", p=128))
    w_sb["wv"] = wv_t
    proj_chunk("wv", vt_raw, 0, 512, tag="aux")
    proj_chunk("wv", vt_raw, 512, 512, tag="sc")
    v_chunk_transpose(0, tag="aux")
    load_xs(1)
    nc.sync.dma_start(out=wo_sb, in_=woT)
    proj_chunk("wk", k_rot, 1024, 512, tag="sc")
    proj_chunk("wk", k_rot, 1536, 512, tag="aux")
    for toff in (1024, 1536):
        rope_chunk(k_rot, toff, 512)

    F = [[] for _ in range(NC_)]
    F[0] = [
        lambda: load_xs(2),
        pj("wv", vt_raw, 1024),
        pj("wv", vt_raw, 1536),
        lambda: v_chunk_transpose(1),
        pj("wq", q_rot, 512),
        rp(q_rot, 512),
    ]
    F[1] = [
        lambda: load_xs(3),
        pj("wq", q_rot, 1024),
        rp(q_rot, 1024),
        pj("wk", k_rot, 2048),
        pj("wk", k_rot, 2560),
    ]
    F[2] = [
        rp(k_rot, 2048),
        rp(k_rot, 2560),
        pj("wq", q_rot, 1536),
        rp(q_rot, 1536),
        pj("wk", k_rot, 3072),
        pj("wk", k_rot, 3584),
    ]
    F[3] = [
        rp(k_rot, 3072),
        rp(k_rot, 3584),
        pj("wv", vt_raw, 2048),
        pj("wv", vt_raw, 2560),
        lambda: v_chunk_transpose(2),
        pj("wq", q_rot, 2048),
        rp(q_rot, 2048),
    ]
    F[4] = [
        pj("wv", vt_raw, 3072),
        pj("wv", vt_raw, 3584),
        lambda: v_chunk_transpose(3),
        pj("wq", q_rot, 2560),
        rp(q_rot, 2560),
    ]
    F[5] = [pj("wq", q_rot, 3072), rp(q_rot, 3072)]
    F[6] = [pj("wq", q_rot, 3584), rp(q_rot, 3584)]

    # flat step loop with cross-boundary score lookahead
    TOT = NC_ * KT_N
    fillers = list(F[0])
    s_exp(0, 0)
    s_exp(0, 1)
    norms = {}
    for step in range(TOT):
        ci, kt = divmod(step, KT_N)
        if kt == 0 and ci > 0:
            # new chunk's deadline pieces right after the pending norm
            if fillers and fillers[0] is norms.get(ci - 1):
                fillers = fillers[:1] + list(F[ci]) + fillers[1:]
            else:
                fillers = list(F[ci]) + fillers
        nxt = step + 2
        if nxt < TOT:
            s_exp(*divmod(nxt, KT_N))
        av(ci, kt)
        if kt == KT_N - 1:
            norms[ci] = make_norm(ci)
            if ci + 1 < NC_:
                fillers.insert(0, norms[ci])
                fillers.extend(oproj_piece(ci, j) for j in range(4))
        if fillers and kt >= 1:
            fillers.pop(0)()
    while fillers:
        fillers.pop(0)()

    # tail: last chunk's normalization + output projection (alternating PSUM
    # tags so the pieces pipeline)
    norms[NC_ - 1]()
    for j in range(4):
        oproj_piece(NC_ - 1, j, tag=("aux", "sc")[j % 2])()

    pp.release()
    at.release()
    const.release()
    psum.release()


_NC_CACHE = {}


def _build_program():
    if 0 in _NC_CACHE:
        return _NC_CACHE[0]
    nc = bacc.Bacc("TRN2", num_devices=N_CORES, debug=False)
    xT = nc.dram_tensor("xT", [D, NTOK], F16, kind="ExternalInput").ap()
    wqT = nc.dram_tensor("wqT", [D, 128], F16, kind="ExternalInput").ap()
    wkT = nc.dram_tensor("wkT", [D, 128], F16, kind="ExternalInput").ap()
    wvT = nc.dram_tensor("wvT", [D, 128], F16, kind="ExternalInput").ap()
    woT = nc.dram_tensor("woT", [128, D], F16, kind="ExternalInput").ap()
    ropeA = nc.dram_tensor("ropeA", [128, T], F16, kind="ExternalInput").ap()
    ropeB = nc.dram_tensor("ropeB", [128, T], F16, kind="ExternalInput").ap()
    outT = nc.dram_tensor("outT", [D, NTOK], F16, kind="ExternalOutput").ap()
    with tile.TileContext(nc) as tc:
        _build_body(tc, xT, wqT, wkT, wvT, woT, ropeA, ropeB, outT)
    nc.compile()
    _NC_CACHE[0] = nc
    return nc


def _rope_tables():
    half = DK // 2  # 32
    inv_freq = 1.0 / (
        10000.0 ** (np.arange(0, DK, 2, dtype=np.float32) / np.float32(DK))
    )
    t = np.arange(T, dtype=np.float32)
    freqs = np.outer(t, inv_freq)  # [T, 32]
    cos = np.cos(freqs)
    sin = np.sin(freqs)
    A = np.empty((128, T), np.float32)
    Bt = np.empty((128, T), np.float32)
    for p in range(128):
        i = p % DK
        if i < half:
            a, bb = cos[:, i], -sin[:, i]
        else:
            a, bb = cos[:, i - half], sin[:, i - half]
        A[p, :] = a
        Bt[p, :] = bb
    return A.astype(np.float16), Bt.astype(np.float16)


def _prep_inputs(x, wq, wk, wv, wo):
    xT = np.ascontiguousarray(x.reshape(NTOK, D).T).astype(np.float16)
    ropeA, ropeB = _rope_tables()
    in_maps = []
    for c in range(N_CORES):
        rows = slice(128 * c, 128 * (c + 1))
        in_maps.append(
            {
                "xT": xT,
                "wqT": np.ascontiguousarray(wq[rows, :].T).astype(np.float16),
                "wkT": np.ascontiguousarray(wk[rows, :].T).astype(np.float16),
                "wvT": np.ascontiguousarray(wv[rows, :].T).astype(np.float16),
                "woT": np.ascontiguousarray(wo[:, rows].T).astype(np.float16),
                "ropeA": ropeA,
                "ropeB": ropeB,
            }
        )
    return in_maps


def run(x, wq, wk, wv, wo, trace=False):
    """Returns (output (B,T,D) fp32, BassKernelResults)."""
    from concourse import bass_utils

    nc = _build_program()
    in_maps = _prep_inputs(
        np.asarray(x, np.float32),
        np.asarray(wq, np.float32),
        np.asarray(wk, np.float32),
        np.asarray(wv, np.float32),
        np.asarray(wo, np.float32),
    )
    res = bass_utils.run_bass_kernel_spmd(
        nc, in_maps, core_ids=list(range(N_CORES)), trace=trace
    )
    acc = np.zeros((D, NTOK), np.float32)
    for c in range(N_CORES):
        acc += np.asarray(res.results[c]["outT"], np.float32)
    out = acc.T.reshape(B, T, D)
    return out, res


def kernel(x, wq, wk, wv, wo):
    out, _ = run(x, wq, wk, wv, wo)
    return out
